# revision 8
# baseline (speedup 1.0000x reference)
"""AdaConv (per-sample dynamic grouped conv) on 8 TRN2 NeuronCores.

Data parallel: batch 16 -> 8 cores x 2 samples. Per core:
  - predictor convs (tiny bf16 matmuls) produce per-sample dw (512,8,3,3)
    and pw (512,8) kernels + bias (512,)
  - the pointwise conv is folded into the depthwise weights on device:
    V[k] = P @ W[k] per group (exact; no activation between dw+pw convs),
    via 9 small matmuls whose block-diag structure comes from the masked
    pwlT operand; V lands directly in conv-stationary layout
  - instance norm (DVE) + reflect pad into a [128, 66, 66] bf16 tile;
    cn also written contiguously into the f32 out buffer (gpsimd)
  - 3x3 grouped conv = 9 shifted-AP bf16 matmuls accumulated in PSUM =
    full dynamic conv; LeakyRelu(+bias) on ACT; in-place += into obuf
    (DVE/gpsimd alternating); one output DMA per (n, ts)
Channel tiling: 512 ch = 4 tiles x 128 partitions; group g (8ch) is tile-local.
"""

import numpy as np

N_FULL = 16
N_CORES = 8
NPC = 2            # samples per core
TS = 4             # channel tiles of 128
CH = 512
HW = 64
SP = HW * HW       # 4096
PADW = HW + 2      # 66
EPS = 1e-5
NCHUNK = 8         # spatial chunks of 512 px (8 rows)
CK = SP // NCHUNK  # 512


def _host_prep(style_encoding, content_in, dw_w, dw_b, pw_kn_w, pw_kn_b,
               pw_bias_w, pw_bias_b):
    """Layout-only transforms. Returns list of 8 per-core input dicts."""
    import ml_dtypes
    bf16 = ml_dtypes.bfloat16

    # dw predictor: output partition m = (G, o) = dw OUTPUT channel; the
    # 8 per-ts iterations enumerate ic i. (Fold-ready layout.)
    ts_i = np.arange(TS)[:, None, None]
    i_i = np.arange(8)[None, :, None]
    m_i = np.arange(128)[None, None, :]
    OCd = 8 * (128 * ts_i + m_i) + i_i               # (4,8i,128m)
    W = dw_w[OCd]                                    # (4,8,128m,64ic,2,2)
    Wk = np.transpose(W, (0, 1, 4, 5, 3, 2)).reshape(TS, 8, 4, 64, 128)
    dwp = np.zeros((TS, 8, 4, 128, 128), np.float32)
    dwp[:, :, :, 0:64, 0:64] = Wk[:, :, :, :, 0:64]
    dwp[:, :, :, 64:128, 64:128] = Wk[:, :, :, :, 64:128]
    dwp = dwp.reshape(32, 4, 128, 128)               # (mt, koff, r, m)
    dwp = np.ascontiguousarray(dwp.transpose(2, 0, 1, 3)).astype(bf16)
    # -> (128r, 32mt, 4koff, 128m)
    p_i = np.arange(128)[:, None, None]
    OC2d = 8 * (128 * np.arange(TS)[None, :, None] + p_i) \
        + np.arange(8)[None, None, :]                # (128p,4,8i)
    dwb = dw_b[OC2d].reshape(128, 32).astype(np.float32)

    # pw predictor: partition = (G, ic); iterations enumerate o'
    o_i = np.arange(8)[None, :, None]
    OC = 8 * (128 * ts_i + 8 * (m_i // 8) + o_i) + (m_i % 8)  # (4,8,128)
    Wp = np.transpose(pw_kn_w[OC, :, 0, 0] / 16.0, (0, 1, 3, 2))  # (4,8,64,128)
    pwp = np.zeros((TS, 8, 128, 128), np.float32)
    pwp[:, :, 0:64, 0:64] = Wp[:, :, :, 0:64]
    pwp[:, :, 64:128, 64:128] = Wp[:, :, :, 64:128]
    pwp = pwp.reshape(32, 128, 128)                  # (mt, r, m)
    pwp = np.ascontiguousarray(pwp.transpose(1, 0, 2)).astype(bf16)
    # -> (128r, 32mt, 128m)
    OC2 = 8 * (128 * np.arange(TS)[None, :, None] + 8 * (p_i // 8)
               + np.arange(8)[None, None, :]) + (p_i % 8)        # (128,4,8)
    pwkb = pw_kn_b[OC2].reshape(128, 32).astype(np.float32)

    pwbT = np.ascontiguousarray(
        (pw_bias_w[:, :, 0, 0].T / 16.0).reshape(TS, 128, CH)
        .transpose(1, 0, 2)).astype(bf16)                        # (128,4,512)
    pbb = np.ascontiguousarray(pw_bias_b.reshape(TS, 128).T)     # (128,4)
    mask_h = (np.arange(128)[:, None] // 8
              == np.arange(16)[None, :]).astype(np.float32)      # (128,16)

    in_maps = []
    for c in range(N_CORES):
        sl = slice(c * NPC, (c + 1) * NPC)
        style_core = np.asarray(style_encoding[sl])              # (2,512,4,4)
        sd = style_core.reshape(NPC, TS, 128, 4, 4).transpose(2, 1, 0, 3, 4)
        content_core = np.ascontiguousarray(
            np.asarray(content_in[sl]).reshape(NPC, TS, 128, SP))
        in_maps.append({
            "style": np.ascontiguousarray(sd).astype(bf16),
            "content": content_core.astype(bf16),
            "dwp": dwp, "pwp": pwp, "pwbT": pwbT,
            "dwb": dwb, "pwkb": pwkb, "pbb": pbb,
            "mask": mask_h,
        })
    return in_maps


def _build_nc():
    import concourse.bass as bass
    import concourse.mybir as mybir
    from concourse import bacc
    from concourse.tile import TileContext

    f32, bf = mybir.dt.float32, mybir.dt.bfloat16
    AF = mybir.ActivationFunctionType
    ALU = mybir.AluOpType
    AX = mybir.AxisListType

    nc = bacc.Bacc()
    style_d = nc.declare_dram_parameter("style", [128, TS, NPC, 4, 4], bf, False)
    content_d = nc.declare_dram_parameter("content", [NPC, TS, 128, SP], bf, False)
    dwp_d = nc.declare_dram_parameter("dwp", [128, 32, 4, 128], bf, False)
    pwp_d = nc.declare_dram_parameter("pwp", [128, 32, 128], bf, False)
    pwbT_d = nc.declare_dram_parameter("pwbT", [128, TS, CH], bf, False)
    dwb_d = nc.declare_dram_parameter("dwb", [128, 32], f32, False)
    pwkb_d = nc.declare_dram_parameter("pwkb", [128, 32], f32, False)
    pbb_d = nc.declare_dram_parameter("pbb", [128, TS], f32, False)
    mask_d = nc.declare_dram_parameter("mask", [128, 16], f32, False)
    out_d = nc.declare_dram_parameter("out", [NPC, TS, 128, SP], f32, True)

    with TileContext(nc) as tc:
        with (
            tc.tile_pool(name="persist", bufs=1) as pp,
            tc.tile_pool(name="wx", bufs=2) as wxp,
            tc.tile_pool(name="cts", bufs=2) as ctp,
            tc.tile_pool(name="pads", bufs=3) as padp,
            tc.tile_pool(name="work", bufs=3) as wkp,
            tc.tile_pool(name="obufs", bufs=3) as obp,
            tc.tile_pool(name="stats", bufs=4) as stp,
            tc.tile_pool(name="psA", bufs=2, space="PSUM") as psa,
            tc.tile_pool(name="psV", bufs=2, space="PSUM") as psv,
            tc.tile_pool(name="psD", bufs=3, space="PSUM") as psd,
        ):
            style_sb = pp.tile([128, TS, NPC, 4, 4], bf, tag="style")
            dwb_sb = pp.tile([128, 32], f32, tag="dwb")
            pwkb_sb = pp.tile([128, 32], f32, tag="pwkb")
            pbb_sb = pp.tile([128, TS], f32, tag="pbb")
            pwbT_sb = pp.tile([128, TS, CH], bf, tag="pwbT")
            dwp_sb = pp.tile([128, 32, 4, 128], bf, tag="dwp")
            pwp_sb = pp.tile([128, 32, 128], bf, tag="pwp")
            sd_f = pp.tile([128, TS, NPC], f32, tag="sdf")
            sd_sb = pp.tile([128, TS, NPC], bf, tag="sd")
            V_sb = pp.tile([128, TS, NPC, 9, 128], bf, tag="Vsb")
            pwlT = pp.tile([128, TS, NPC, 16, 8], bf, tag="pwlT")
            S_dw = pp.tile([128, TS, NPC, 9, 8], f32, tag="Sdw")
            S_pw = pp.tile([128, TS, NPC, 8], f32, tag="Spw")
            bias_sb = pp.tile([128, TS, NPC], f32, tag="bias")
            eps_sb = pp.tile([128, 1], f32, tag="eps")
            mask = pp.tile([128, 16], f32, tag="mask")
            sqscr = pp.tile([128, SP], bf, tag="sqscr")
            nc.vector.memset(eps_sb[:], EPS)

            # ---- head DMAs: weights for ts0 first, then content, then rest
            nc.sync.dma_start(out=mask[:], in_=mask_d[:])
            nc.sync.dma_start(
                out=style_sb[:].rearrange("p a n y x -> p (a n y x)"),
                in_=style_d[:].rearrange("p a n y x -> p (a n y x)"))
            nc.sync.dma_start(out=dwb_sb[:], in_=dwb_d[:])
            nc.sync.dma_start(out=pwkb_sb[:], in_=pwkb_d[:])
            nc.sync.dma_start(out=pbb_sb[:], in_=pbb_d[:])

            def wdma(ts):
                nc.sync.dma_start(
                    out=dwp_sb[:, 8 * ts:8 * (ts + 1), :, :].rearrange(
                        "p a k m -> p (a k m)"),
                    in_=dwp_d[:, 8 * ts:8 * (ts + 1), :, :].rearrange(
                        "p a k m -> p (a k m)"))
                nc.sync.dma_start(
                    out=pwp_sb[:, 8 * ts:8 * (ts + 1), :].rearrange(
                        "p a m -> p (a m)"),
                    in_=pwp_d[:, 8 * ts:8 * (ts + 1), :].rearrange(
                        "p a m -> p (a m)"))

            wdma(0)

            # style spatial sum -> sd (x 1/16 folded into pw weights)
            for ts in range(TS):
                nc.vector.tensor_reduce(
                    out=sd_f[:, ts, :], in_=style_sb[:, ts, :, :, :],
                    op=ALU.add, axis=AX.XY)
            nc.vector.tensor_copy(
                sd_sb[:].rearrange("p a n -> p (a n)"),
                sd_f[:].rearrange("p a n -> p (a n)"))

            # ---- instance-norm prologue ----
            def prologue(n, ts):
                ctile = ctp.tile([128, SP], bf, tag="ctile")
                nc.sync.dma_start(out=ctile[:], in_=content_d[n, ts])
                pad = padp.tile([128, PADW, PADW], bf, tag="pad")
                interior = pad[:, 1:65, 1:65]
                obuf = obp.tile([128, SP], f32, tag="obuf")

                s_t = stp.tile([128, 1], f32, tag="sum")
                sq_t = stp.tile([128, 1], f32, tag="sumsq")
                nc.vector.tensor_reduce(out=s_t[:], in_=ctile[:],
                                        op=ALU.add, axis=AX.X)
                nc.vector.tensor_tensor(out=sqscr[:], in0=ctile[:],
                                        in1=ctile[:], op=ALU.mult)
                nc.vector.tensor_reduce(out=sq_t[:], in_=sqscr[:],
                                        op=ALU.add, axis=AX.X)
                mean = stp.tile([128, 1], f32, tag="mean")
                nc.vector.tensor_scalar(out=mean[:], in0=s_t[:],
                                        scalar1=1.0 / SP, scalar2=None,
                                        op0=ALU.mult)
                msq = stp.tile([128, 1], f32, tag="msq")
                nc.vector.scalar_tensor_tensor(
                    out=msq[:], in0=mean[:], scalar=1.0, in1=s_t[:],
                    op0=ALU.mult, op1=ALU.mult)
                var_ = stp.tile([128, 1], f32, tag="var")
                nc.vector.scalar_tensor_tensor(
                    out=var_[:], in0=sq_t[:], scalar=1.0, in1=msq[:],
                    op0=ALU.mult, op1=ALU.subtract)
                std = stp.tile([128, 1], f32, tag="std")
                nc.scalar.activation(std[:], var_[:], AF.Sqrt,
                                     bias=eps_sb[:, 0:1],
                                     scale=1.0 / (SP - 1))
                rstd = stp.tile([128, 1], f32, tag="rstd")
                nc.vector.reciprocal(rstd[:], std[:])
                nshift = stp.tile([128, 1], f32, tag="nshift")
                nc.vector.scalar_tensor_tensor(
                    out=nshift[:], in0=mean[:], scalar=-1.0, in1=rstd[:],
                    op0=ALU.mult, op1=ALU.mult)
                nc.vector.tensor_scalar(
                    out=interior,
                    in0=ctile[:].rearrange("p (a b) -> p a b", a=HW),
                    scalar1=rstd[:, 0:1], scalar2=nshift[:, 0:1],
                    op0=ALU.mult, op1=ALU.add)
                # contiguous f32 copy of cn for the in-place residual adds
                nc.gpsimd.tensor_scalar(
                    out=obuf[:], in0=ctile[:],
                    scalar1=rstd[:, 0:1], scalar2=nshift[:, 0:1],
                    op0=ALU.mult, op1=ALU.add)
                nc.vector.tensor_copy(pad[:, 0, 1:65], pad[:, 2, 1:65])
                nc.vector.tensor_copy(pad[:, 65, 1:65], pad[:, 63, 1:65])
                nc.vector.tensor_copy(pad[:, :, 0], pad[:, :, 2])
                nc.vector.tensor_copy(pad[:, :, 65], pad[:, :, 63])
                return pad, obuf

            order = [(n, ts) for n in range(NPC) for ts in range(TS)]
            pending = {order[0]: prologue(*order[0])}
            wdma(1)
            pending[order[1]] = prologue(*order[1])
            wdma(2)
            wdma(3)
            nc.sync.dma_start(
                out=pwbT_sb[:].rearrange("p a b -> p (a b)"),
                in_=pwbT_d[:].rearrange("p a b -> p (a b)"))

            # ---- predictor + fold, per ts ----
            def pred(ts):
                for i in range(8):
                    mt = ts * 8 + i
                    ps = psa.tile([128, NPC, 3, 3], f32, tag="psA")
                    for koff in range(4):
                        ky, kx = divmod(koff, 2)
                        nc.tensor.matmul(
                            ps[:], dwp_sb[:, mt, koff, :],
                            style_sb[:, ts, :, ky:ky + 3, kx:kx + 3],
                            start=(koff == 0), stop=(koff == 3))
                    nc.scalar.activation(
                        S_dw[:, ts, :, :, i], ps[:], AF.Relu,
                        bias=dwb_sb[:, mt:mt + 1])

                    ps2 = psa.tile([128, NPC], f32, tag="psA")
                    nc.tensor.matmul(ps2[:], pwp_sb[:, mt, :], sd_sb[:, ts, :],
                                     start=True, stop=True)
                    nc.scalar.activation(
                        S_pw[:, ts, :, i], ps2[:], AF.Relu,
                        bias=pwkb_sb[:, mt:mt + 1])
                # expansions: Wx (fold stationary) + pwlT (fold moving)
                Wx = wxp.tile([128, NPC, 9, 16, 8], bf, tag="Wx")
                for G in range(16):
                    nc.vector.tensor_scalar(
                        out=Wx[:, :, :, G, :], in0=S_dw[:, ts],
                        scalar1=mask[:, G:G + 1], scalar2=None, op0=ALU.mult)
                    nc.vector.tensor_scalar(
                        out=pwlT[:, ts, :, G, :], in0=S_pw[:, ts],
                        scalar1=mask[:, G:G + 1], scalar2=None, op0=ALU.mult)
                # fold: V[k] = P @ W[k] per group, directly in stationary
                # layout (partition=(G,i), m=(G,o')); cross-group zeros come
                # from pwlT's block-diagonality
                for n in range(NPC):
                    pwT = pwlT[:, ts, n].rearrange("p g o -> p (g o)")
                    for kb in range(3):
                        psv_t = psv.tile([128, 3, 128], f32, tag="psV")
                        for j in range(3):
                            k = kb * 3 + j
                            nc.tensor.matmul(
                                psv_t[:, j, :],
                                Wx[:, n, k].rearrange("p g o -> p (g o)"),
                                pwT, start=True, stop=True)
                        nc.scalar.activation(
                            V_sb[:, ts, n, 3 * kb:3 * kb + 3, :],
                            psv_t[:], AF.Copy)

            pred(0)
            # dynamic pointwise bias (needed by first conv postprocessing)
            for ts in range(TS):
                ps3 = psa.tile([128, NPC], f32, tag="psA")
                for kt in range(TS):
                    nc.tensor.matmul(
                        ps3[:], pwbT_sb[:, kt, 128 * ts:128 * (ts + 1)],
                        sd_sb[:, kt, :], start=(kt == 0), stop=(kt == 3))
                nc.scalar.activation(bias_sb[:, ts, :], ps3[:], AF.Relu,
                                     bias=pbb_sb[:, ts:ts + 1])
            pred(1)
            pred(2)
            pred(3)

            # ---- main conv per (n, ts): 8 chunks of 8 rows ----
            def conv(n, ts, pad, obuf):
                for c in range(NCHUNK):
                    ps4 = psd.tile([128, 8, 64], f32, tag="psD")
                    for k in range(9):
                        dy, dx = divmod(k, 3)
                        nc.tensor.matmul(
                            ps4[:], V_sb[:, ts, n, k, :],
                            pad[:, c * 8 + dy:c * 8 + dy + 8,
                                dx:dx + 64],
                            start=(k == 0), stop=(k == 8))
                    ot = wkp.tile([128, CK], f32, tag="ot")
                    nc.scalar.activation(
                        ot[:].rearrange("p (a b) -> p a b", a=8), ps4[:],
                        AF.Lrelu, bias=bias_sb[:, ts, n:n + 1], alpha=0.01)
                    och = obuf[:, c * CK:(c + 1) * CK]
                    eng = nc.vector if c % 2 == 0 else nc.gpsimd
                    eng.tensor_tensor(out=och, in0=och, in1=ot[:], op=ALU.add)
                nc.gpsimd.dma_start(out=out_d[n, ts], in_=obuf[:])

            for i, key in enumerate(order):
                if i + 2 < len(order):
                    pending[order[i + 2]] = prologue(*order[i + 2])
                conv(key[0], key[1], *pending.pop(key))
    nc.compile()
    return nc


_NC_CACHE = None


def kernel(**inputs):
    global _NC_CACHE
    in_maps = _host_prep(**inputs)
    if _NC_CACHE is None:
        _NC_CACHE = _build_nc()
    nc = _NC_CACHE
    from concourse.bass_utils import run_bass_kernel_spmd
    res = run_bass_kernel_spmd(nc, in_maps, core_ids=list(range(N_CORES)))
    outs = []
    for c in range(N_CORES):
        o = res.results[c]["out"].reshape(NPC, TS, 128, SP)
        outs.append(o.reshape(NPC, CH, HW, HW))
    return np.concatenate(outs, axis=0).astype(np.float32)


# revision 15
# speedup vs baseline: 1.1744x; 1.1744x over previous
"""AdaConv (per-sample dynamic grouped conv) on 8 TRN2 NeuronCores.

Data parallel: batch 16 -> 8 cores x 2 samples. Per core:
  - predictor convs (tiny bf16 matmuls) produce per-sample dw (512,8,3,3)
    and pw (512,8) kernels + bias (512,)
  - the pointwise conv is folded into the depthwise weights on device:
    V[k] = P @ W[k] per group (exact; no activation between dw+pw convs),
    via 9 small matmuls whose block-diag structure comes from the masked
    pwlT operand; V lands directly in conv-stationary layout
  - instance norm (DVE) + reflect pad into a [128, 66, 66] bf16 tile;
    cn also written contiguously into the f32 out buffer (gpsimd)
  - 3x3 grouped conv = 9 shifted-AP bf16 matmuls accumulated in PSUM =
    full dynamic conv; LeakyRelu(+bias) on ACT; in-place += into obuf
    (DVE/gpsimd alternating); one output DMA per (n, ts)
Channel tiling: 512 ch = 4 tiles x 128 partitions; group g (8ch) is tile-local.
"""

import numpy as np

N_FULL = 16
N_CORES = 8
NPC = 2            # samples per core
TS = 4             # channel tiles of 128
CH = 512
HW = 64
SP = HW * HW       # 4096
PADW = HW + 2      # 66
EPS = 1e-5
NCHUNK = 8         # spatial chunks of 512 px (8 rows)
CK = SP // NCHUNK  # 512


def _host_prep(style_encoding, content_in, dw_w, dw_b, pw_kn_w, pw_kn_b,
               pw_bias_w, pw_bias_b):
    """Layout-only transforms. Returns list of 8 per-core input dicts."""
    import ml_dtypes
    bf16 = ml_dtypes.bfloat16

    # dw predictor: output partition m = (G, o) = dw OUTPUT channel; the
    # 8 per-ts iterations enumerate ic i. (Fold-ready layout.)
    ts_i = np.arange(TS)[:, None, None]
    i_i = np.arange(8)[None, :, None]
    m_i = np.arange(128)[None, None, :]
    OCd = 8 * (128 * ts_i + m_i) + i_i               # (4,8i,128m)
    W = dw_w[OCd]                                    # (4,8,128m,64ic,2,2)
    Wk = np.transpose(W, (0, 1, 4, 5, 3, 2)).reshape(TS, 8, 4, 64, 128)
    dwp = np.zeros((TS, 8, 4, 128, 128), np.float32)
    dwp[:, :, :, 0:64, 0:64] = Wk[:, :, :, :, 0:64]
    dwp[:, :, :, 64:128, 64:128] = Wk[:, :, :, :, 64:128]
    dwp = dwp.reshape(32, 4, 128, 128)               # (mt, koff, r, m)
    dwp = np.ascontiguousarray(dwp.transpose(2, 0, 1, 3)).astype(bf16)
    # -> (128r, 32mt, 4koff, 128m)
    p_i = np.arange(128)[:, None, None]
    OC2d = 8 * (128 * np.arange(TS)[None, :, None] + p_i) \
        + np.arange(8)[None, None, :]                # (128p,4,8i)
    dwb = dw_b[OC2d].reshape(128, 32).astype(np.float32)

    # pw predictor: partition = (G, ic); iterations enumerate o'
    o_i = np.arange(8)[None, :, None]
    OC = 8 * (128 * ts_i + 8 * (m_i // 8) + o_i) + (m_i % 8)  # (4,8,128)
    Wp = np.transpose(pw_kn_w[OC, :, 0, 0] / 16.0, (0, 1, 3, 2))  # (4,8,64,128)
    pwp = np.zeros((TS, 8, 128, 128), np.float32)
    pwp[:, :, 0:64, 0:64] = Wp[:, :, :, 0:64]
    pwp[:, :, 64:128, 64:128] = Wp[:, :, :, 64:128]
    pwp = pwp.reshape(32, 128, 128)                  # (mt, r, m)
    pwp = np.ascontiguousarray(pwp.transpose(1, 0, 2)).astype(bf16)
    # -> (128r, 32mt, 128m)
    OC2 = 8 * (128 * np.arange(TS)[None, :, None] + 8 * (p_i // 8)
               + np.arange(8)[None, None, :]) + (p_i % 8)        # (128,4,8)
    pwkb = pw_kn_b[OC2].reshape(128, 32).astype(np.float32)

    pwbT = np.ascontiguousarray(
        (pw_bias_w[:, :, 0, 0].T / 16.0).reshape(TS, 128, CH)
        .transpose(1, 0, 2)).astype(bf16)                        # (128,4,512)
    pbb = np.ascontiguousarray(pw_bias_b.reshape(TS, 128).T)     # (128,4)
    mask_h = (np.arange(128)[:, None] // 8
              == np.arange(16)[None, :]).astype(np.float32)      # (128,16)

    in_maps = []
    for c in range(N_CORES):
        sl = slice(c * NPC, (c + 1) * NPC)
        style_core = np.asarray(style_encoding[sl])              # (2,512,4,4)
        sd = style_core.reshape(NPC, TS, 128, 4, 4).transpose(2, 1, 0, 3, 4)
        content_core = np.ascontiguousarray(
            np.asarray(content_in[sl]).reshape(NPC, TS, 128, SP))
        in_maps.append({
            "style": np.ascontiguousarray(sd).astype(bf16),
            "content": content_core.astype(bf16),
            "dwp": dwp, "pwp": pwp, "pwbT": pwbT,
            "dwb": dwb, "pwkb": pwkb, "pbb": pbb,
            "mask": mask_h,
        })
    return in_maps


def _build_nc():
    import concourse.bass as bass
    import concourse.mybir as mybir
    from concourse import bacc
    from concourse.tile import TileContext

    f32, bf = mybir.dt.float32, mybir.dt.bfloat16
    AF = mybir.ActivationFunctionType
    ALU = mybir.AluOpType
    AX = mybir.AxisListType

    nc = bacc.Bacc()
    style_d = nc.declare_dram_parameter("style", [128, TS, NPC, 4, 4], bf, False)
    content_d = nc.declare_dram_parameter("content", [NPC, TS, 128, SP], bf, False)
    dwp_d = nc.declare_dram_parameter("dwp", [128, 32, 4, 128], bf, False)
    pwp_d = nc.declare_dram_parameter("pwp", [128, 32, 128], bf, False)
    pwbT_d = nc.declare_dram_parameter("pwbT", [128, TS, CH], bf, False)
    dwb_d = nc.declare_dram_parameter("dwb", [128, 32], f32, False)
    pwkb_d = nc.declare_dram_parameter("pwkb", [128, 32], f32, False)
    pbb_d = nc.declare_dram_parameter("pbb", [128, TS], f32, False)
    mask_d = nc.declare_dram_parameter("mask", [128, 16], f32, False)
    out_d = nc.declare_dram_parameter("out", [NPC, TS, 128, SP], f32, True)

    with TileContext(nc) as tc:
        with (
            tc.tile_pool(name="persist", bufs=1) as pp,
            tc.tile_pool(name="wx", bufs=2) as wxp,
            tc.tile_pool(name="cts", bufs=3) as ctp,
            tc.tile_pool(name="pads", bufs=3) as padp,
            tc.tile_pool(name="work", bufs=3) as wkp,
            tc.tile_pool(name="obufs", bufs=2) as obp,
            tc.tile_pool(name="stats", bufs=4) as stp,
            tc.tile_pool(name="psA", bufs=2, space="PSUM") as psa,
            tc.tile_pool(name="psV", bufs=2, space="PSUM") as psv,
            tc.tile_pool(name="psD", bufs=3, space="PSUM") as psd,
        ):
            style_sb = pp.tile([128, TS, NPC, 4, 4], bf, tag="style")
            dwb_sb = pp.tile([128, 32], f32, tag="dwb")
            pwkb_sb = pp.tile([128, 32], f32, tag="pwkb")
            pbb_sb = pp.tile([128, TS], f32, tag="pbb")
            pwbT_sb = pp.tile([128, TS, CH], bf, tag="pwbT")
            dwp_sb = pp.tile([128, 32, 4, 128], bf, tag="dwp")
            pwp_sb = pp.tile([128, 32, 128], bf, tag="pwp")
            sd_f = pp.tile([128, TS, NPC], f32, tag="sdf")
            sd_sb = pp.tile([128, TS, NPC], bf, tag="sd")
            V_sb = pp.tile([128, TS, NPC, 9, 128], bf, tag="Vsb")
            pwlT = pp.tile([128, TS, NPC, 16, 8], bf, tag="pwlT")
            S_dw = pp.tile([128, TS, NPC, 9, 8], f32, tag="Sdw")
            S_pw = pp.tile([128, TS, NPC, 8], f32, tag="Spw")
            bias_sb = pp.tile([128, TS, NPC], f32, tag="bias")
            eps_sb = pp.tile([128, 1], f32, tag="eps")
            mask = pp.tile([128, 16], f32, tag="mask")
            sqscr = pp.tile([128, SP], bf, tag="sqscr")
            nc.vector.memset(eps_sb[:], EPS)

            # ---- head DMAs: weights for ts0 first, then content, then rest
            nc.sync.dma_start(out=mask[:], in_=mask_d[:])
            nc.sync.dma_start(
                out=style_sb[:].rearrange("p a n y x -> p (a n y x)"),
                in_=style_d[:].rearrange("p a n y x -> p (a n y x)"))
            nc.sync.dma_start(out=dwb_sb[:], in_=dwb_d[:])
            nc.sync.dma_start(out=pwkb_sb[:], in_=pwkb_d[:])
            nc.sync.dma_start(out=pbb_sb[:], in_=pbb_d[:])

            def wdma(ts):
                nc.sync.dma_start(
                    out=dwp_sb[:, 8 * ts:8 * (ts + 1), :, :].rearrange(
                        "p a k m -> p (a k m)"),
                    in_=dwp_d[:, 8 * ts:8 * (ts + 1), :, :].rearrange(
                        "p a k m -> p (a k m)"))
                nc.sync.dma_start(
                    out=pwp_sb[:, 8 * ts:8 * (ts + 1), :].rearrange(
                        "p a m -> p (a m)"),
                    in_=pwp_d[:, 8 * ts:8 * (ts + 1), :].rearrange(
                        "p a m -> p (a m)"))

            wdma(0)
            nc.sync.dma_start(
                out=pwbT_sb[:].rearrange("p a b -> p (a b)"),
                in_=pwbT_d[:].rearrange("p a b -> p (a b)"))

            # style spatial sum -> sd (x 1/16 folded into pw weights)
            for ts in range(TS):
                nc.vector.tensor_reduce(
                    out=sd_f[:, ts, :], in_=style_sb[:, ts, :, :, :],
                    op=ALU.add, axis=AX.XY)
            nc.vector.tensor_copy(
                sd_sb[:].rearrange("p a n -> p (a n)"),
                sd_f[:].rearrange("p a n -> p (a n)"))

            # ---- instance-norm prologue ----
            def prologue(n, ts):
                ctile = ctp.tile([128, SP], bf, tag="ctile")
                nc.sync.dma_start(out=ctile[:], in_=content_d[n, ts])
                pad = padp.tile([128, PADW, PADW], bf, tag="pad")
                interior = pad[:, 1:65, 1:65]

                s_t = stp.tile([128, 1], f32, tag="sum")
                sq_t = stp.tile([128, 1], f32, tag="sumsq")
                nc.vector.tensor_reduce(out=s_t[:], in_=ctile[:],
                                        op=ALU.add, axis=AX.X)
                nc.vector.tensor_tensor(out=sqscr[:], in0=ctile[:],
                                        in1=ctile[:], op=ALU.mult)
                nc.vector.tensor_reduce(out=sq_t[:], in_=sqscr[:],
                                        op=ALU.add, axis=AX.X)
                mean = stp.tile([128, 1], f32, tag="mean")
                nc.vector.tensor_scalar(out=mean[:], in0=s_t[:],
                                        scalar1=1.0 / SP, scalar2=None,
                                        op0=ALU.mult)
                msq = stp.tile([128, 1], f32, tag="msq")
                nc.vector.scalar_tensor_tensor(
                    out=msq[:], in0=mean[:], scalar=1.0, in1=s_t[:],
                    op0=ALU.mult, op1=ALU.mult)
                var_ = stp.tile([128, 1], f32, tag="var")
                nc.vector.scalar_tensor_tensor(
                    out=var_[:], in0=sq_t[:], scalar=1.0, in1=msq[:],
                    op0=ALU.mult, op1=ALU.subtract)
                std = stp.tile([128, 1], f32, tag="std")
                nc.scalar.activation(std[:], var_[:], AF.Sqrt,
                                     bias=eps_sb[:, 0:1],
                                     scale=1.0 / (SP - 1))
                rstd = stp.tile([128, 1], f32, tag="rstd")
                nc.vector.reciprocal(rstd[:], std[:])
                nshift = stp.tile([128, 1], f32, tag="nshift")
                nc.vector.scalar_tensor_tensor(
                    out=nshift[:], in0=mean[:], scalar=-1.0, in1=rstd[:],
                    op0=ALU.mult, op1=ALU.mult)
                nc.vector.tensor_scalar(
                    out=interior,
                    in0=ctile[:].rearrange("p (a b) -> p a b", a=HW),
                    scalar1=rstd[:, 0:1], scalar2=nshift[:, 0:1],
                    op0=ALU.mult, op1=ALU.add)
                nc.gpsimd.tensor_copy(pad[:, 0, 1:65], pad[:, 2, 1:65])
                nc.gpsimd.tensor_copy(pad[:, 65, 1:65], pad[:, 63, 1:65])
                nc.gpsimd.tensor_copy(pad[:, :, 0], pad[:, :, 2])
                nc.gpsimd.tensor_copy(pad[:, :, 65], pad[:, :, 63])
                return pad

            order = [(n, ts) for n in range(NPC) for ts in range(TS)]
            pending = {order[0]: prologue(*order[0])}
            wdma(1)
            pending[order[1]] = prologue(*order[1])
            wdma(2)
            wdma(3)

            # ---- predictor + fold, per ts ----
            def pred(ts):
                for i in range(8):
                    mt = ts * 8 + i
                    ps = psa.tile([128, NPC, 3, 3], f32, tag="psA")
                    for koff in range(4):
                        ky, kx = divmod(koff, 2)
                        nc.tensor.matmul(
                            ps[:], dwp_sb[:, mt, koff, :],
                            style_sb[:, ts, :, ky:ky + 3, kx:kx + 3],
                            start=(koff == 0), stop=(koff == 3))
                    nc.scalar.activation(
                        S_dw[:, ts, :, :, i], ps[:], AF.Relu,
                        bias=dwb_sb[:, mt:mt + 1])

                    ps2 = psa.tile([128, NPC], f32, tag="psA")
                    nc.tensor.matmul(ps2[:], pwp_sb[:, mt, :], sd_sb[:, ts, :],
                                     start=True, stop=True)
                    nc.scalar.activation(
                        S_pw[:, ts, :, i], ps2[:], AF.Relu,
                        bias=pwkb_sb[:, mt:mt + 1])
                # expansions: Wx (fold stationary) + pwlT (fold moving)
                Wx = wxp.tile([128, NPC, 9, 16, 8], bf, tag="Wx")
                for G in range(16):
                    nc.vector.tensor_scalar(
                        out=Wx[:, :, :, G, :], in0=S_dw[:, ts],
                        scalar1=mask[:, G:G + 1], scalar2=None, op0=ALU.mult)
                    nc.vector.tensor_scalar(
                        out=pwlT[:, ts, :, G, :], in0=S_pw[:, ts],
                        scalar1=mask[:, G:G + 1], scalar2=None, op0=ALU.mult)
                # fold: V[k] = P @ W[k] per group, directly in stationary
                # layout (partition=(G,i), m=(G,o')); cross-group zeros come
                # from pwlT's block-diagonality
                for n in range(NPC):
                    pwT = pwlT[:, ts, n].rearrange("p g o -> p (g o)")
                    for kb in range(3):
                        psv_t = psv.tile([128, 3, 128], f32, tag="psV")
                        for j in range(3):
                            k = kb * 3 + j
                            nc.tensor.matmul(
                                psv_t[:, j, :],
                                Wx[:, n, k].rearrange("p g o -> p (g o)"),
                                pwT, start=True, stop=True)
                        nc.scalar.activation(
                            V_sb[:, ts, n, 3 * kb:3 * kb + 3, :],
                            psv_t[:], AF.Copy)

            pred(0)
            # dynamic pointwise bias (needed by first conv postprocessing)
            for ts in range(TS):
                ps3 = psa.tile([128, NPC], f32, tag="psA")
                for kt in range(TS):
                    nc.tensor.matmul(
                        ps3[:], pwbT_sb[:, kt, 128 * ts:128 * (ts + 1)],
                        sd_sb[:, kt, :], start=(kt == 0), stop=(kt == 3))
                nc.scalar.activation(bias_sb[:, ts, :], ps3[:], AF.Relu,
                                     bias=pbb_sb[:, ts:ts + 1])
            pred(1)
            pred(2)
            pred(3)

            # ---- main conv per (n, ts): 8 chunks of 8 rows ----
            def conv(n, ts, pad):
                obuf = obp.tile([128, SP], f32, tag="obuf")
                for c in range(NCHUNK):
                    ps4 = psd.tile([128, 8, 64], f32, tag="psD")
                    for k in range(9):
                        dy, dx = divmod(k, 3)
                        nc.tensor.matmul(
                            ps4[:], V_sb[:, ts, n, k, :],
                            pad[:, c * 8 + dy:c * 8 + dy + 8,
                                dx:dx + 64],
                            start=(k == 0), stop=(k == 8))
                    ot = wkp.tile([128, CK], f32, tag="ot")
                    nc.scalar.activation(
                        ot[:].rearrange("p (a b) -> p a b", a=8), ps4[:],
                        AF.Lrelu, bias=bias_sb[:, ts, n:n + 1], alpha=0.01)
                    och = obuf[:, c * CK:(c + 1) * CK]
                    eng = nc.vector if c % 2 == 0 else nc.gpsimd
                    eng.tensor_tensor(
                        out=och.rearrange("p (a b) -> p a b", a=8),
                        in0=ot[:].rearrange("p (a b) -> p a b", a=8),
                        in1=pad[:, 1 + c * 8:1 + c * 8 + 8, 1:65],
                        op=ALU.add)
                nc.gpsimd.dma_start(out=out_d[n, ts], in_=obuf[:])

            for i, key in enumerate(order):
                if i + 2 < len(order):
                    pending[order[i + 2]] = prologue(*order[i + 2])
                conv(key[0], key[1], pending.pop(key))
    nc.compile()
    return nc


_NC_CACHE = None


def kernel(**inputs):
    global _NC_CACHE
    in_maps = _host_prep(**inputs)
    if _NC_CACHE is None:
        _NC_CACHE = _build_nc()
    nc = _NC_CACHE
    from concourse.bass_utils import run_bass_kernel_spmd
    res = run_bass_kernel_spmd(nc, in_maps, core_ids=list(range(N_CORES)))
    outs = []
    for c in range(N_CORES):
        o = res.results[c]["out"].reshape(NPC, TS, 128, SP)
        outs.append(o.reshape(NPC, CH, HW, HW))
    return np.concatenate(outs, axis=0).astype(np.float32)
